# revision 23
# baseline (speedup 1.0000x reference)
"""BatchHardTripletLoss kernel for 8 Trainium2 NeuronCores.

Math (matches the jax reference):
  dist2[i,j] = |e1_i|^2 + |e2_j|^2 - 2 e1.e2 + 2*eps*(s1_i - s2_j) + D*eps^2
             = a[i] + (b[j] - 2*G[i,j])
  pos_max[i] = sqrt(clip(a[i] + max_{j in pos}(b[j] - 2 G[i,j]), 0))
  neg_min[i] = sqrt(clip(a[i] + min_{j in neg}(b[j] - 2 G[i,j]), 0))
  loss = mean over pos anchors of relu(pos_max - neg_min + margin)

Device strategy (data parallel over emb1 rows, hint-compliant):
  - Host: sort emb2 rows so target==1 rows come first (masks become
    contiguous column ranges), precompute a[i]/b[j] row stats, transpose
    both embeddings to [D=128, rows] layout, cast to bf16 (rel err of the
    final loss ~1e-5, verified), scale emb1 by -2.
  - Each core gets 1024 anchor rows: computes G-blocks on TensorE
    (bf16, K=128, N=512 per matmul into fp32 PSUM) and reduces with the
    fused DVE op tensor_tensor_reduce: accum = reduce_minmax(psum + bias)
    chained across column groups via the scalar initial value.
  - Device output per core: [128, 16] fp32 = per-i-tile max/min partials.
  - Host: adds a[i], sqrt, margin/relu, weighted mean (O(B) work).
"""

import os
import sys

for _p in ("/opt/trn_rl_repo",):
    if _p not in sys.path:
        sys.path.insert(0, _p)

import numpy as np
import ml_dtypes

EPS = 1e-6
MARGIN = 0.2
B = 8192
D = 128
NCORES = 8
SH = B // NCORES      # anchors per core
NIT = SH // 128       # i-tiles of 128 anchors per core
GW = 2048             # candidate-group width = 4 PSUM banks
NG = B // GW
PSUM_BUFS = 2
BIG = 1.0e30

_programs = {}
LAST_RESULTS = None   # BassKernelResults of the most recent run (for profiling)


def _build_program(k: int):
    """Bass program for one core; k = number of positive candidates
    (boundary between the max- and min-reduced column ranges)."""
    import concourse.bacc as bacc
    import concourse.tile as tile
    from concourse import mybir

    f32 = mybir.dt.float32
    bf16 = mybir.dt.bfloat16
    AOT = mybir.AluOpType

    nc = bacc.Bacc(None)
    e1t = nc.declare_dram_parameter("e1t", [D, SH], bf16, isOutput=False)
    e2t = nc.declare_dram_parameter("e2t", [D, B], bf16, isOutput=False)
    tailw = nc.declare_dram_parameter("tailw", [16, SH], bf16, isOutput=False)
    trhs = nc.declare_dram_parameter("trhs", [16, B], bf16, isOutput=False)
    outp = nc.declare_dram_parameter("out", [128, 2 * NIT], f32, isOutput=True)

    # per-group reduction segments: (lo, hi, is_pos) in global column coords
    def group_segs(g):
        glo, ghi = g * GW, (g + 1) * GW
        segs = []
        if glo < k:
            segs.append((glo, min(ghi, k), True))
        if ghi > k:
            segs.append((max(glo, k), ghi, False))
        return segs

    n_pos_segs = sum(1 for g in range(NG) for s in group_segs(g) if s[2])
    n_neg_segs = sum(1 for g in range(NG) for s in group_segs(g) if not s[2])

    with tile.TileContext(nc) as tc:
        with (
            tc.tile_pool(name="const", bufs=1) as cpool,
            tc.tile_pool(name="e2p", bufs=NG) as e2pool,
            tc.tile_pool(name="ps", bufs=PSUM_BUFS, space="PSUM") as pspool,
            tc.tile_pool(name="red", bufs=4) as redpool,
        ):
            e1sb = cpool.tile([D, SH], bf16, tag="e1sb")
            nc.sync.dma_start(e1sb[:], e1t[:])
            twsb = cpool.tile([128, SH], bf16, tag="twsb")
            trsb = cpool.tile([128, B], bf16, tag="trsb")
            for s in range(4):
                nc.sync.dma_start(twsb[32 * s:32 * s + 4, :], tailw[4 * s:4 * s + 4, :])
            outsb = cpool.tile([128, 2 * NIT], f32, tag="outsb")

            e2sb = []
            for g in range(NG):
                e2c = e2pool.tile([D, GW], bf16, tag="e2c")
                nc.sync.dma_start(e2c[:], e2t[:, g * GW:(g + 1) * GW])
                gsl = slice(g * GW, (g + 1) * GW)
                for s in range(4):
                    nc.sync.dma_start(
                        trsb[32 * s:32 * s + 4, gsl], trhs[4 * s:4 * s + 4, gsl]
                    )
                e2sb.append(e2c)

            for it in range(NIT):
                icols = slice(it * 128, (it + 1) * 128)
                w = e1sb[0:126, icols]
                posb = redpool.tile([128, n_pos_segs], f32, tag="posb")
                negb = redpool.tile([128, n_neg_segs], f32, tag="negb")
                ip = 0
                ineg = 0
                for g in range(NG):
                    ps = pspool.tile([128, GW], f32, tag="ps")
                    # K=126 mains (embedding dims 0..125)
                    for s in range(GW // 512):
                        nc.tensor.matmul(
                            ps[:, s * 512:(s + 1) * 512],
                            w,
                            e2sb[g][0:126, s * 512:(s + 1) * 512],
                            start=True,
                            stop=False,
                        )
                    # K=4 tails (dims 126,127 + bias hi/lo), 4-way
                    # row-packed so the four sub-tiles run concurrently
                    for s in range(GW // 512):
                        j0 = g * GW + s * 512
                        nc.tensor.matmul(
                            ps[:, s * 512:(s + 1) * 512],
                            twsb[32 * s:32 * s + 4, icols],
                            trsb[32 * s:32 * s + 4, j0:j0 + 512],
                            start=False,
                            stop=True,
                            tile_position=(32 * s, 0),
                        )
                    for lo, hi, is_pos in group_segs(g):
                        if is_pos:
                            dst = posb[:, ip:ip + 1]
                            ip += 1
                        else:
                            dst = negb[:, ineg:ineg + 1]
                            ineg += 1
                        nc.vector.tensor_reduce(
                            dst,
                            ps[:, lo - g * GW:hi - g * GW],
                            axis=mybir.AxisListType.X,
                            op=AOT.max if is_pos else AOT.min,
                        )
                nc.vector.tensor_reduce(
                    outsb[:, it:it + 1], posb[:],
                    axis=mybir.AxisListType.X, op=AOT.max,
                )
                nc.vector.tensor_reduce(
                    outsb[:, NIT + it:NIT + it + 1], negb[:],
                    axis=mybir.AxisListType.X, op=AOT.min,
                )
            nc.sync.dma_start(outp[:], outsb[:])
    nc.compile()
    return nc


def _host_prep(emb1, emb2, target):
    tpos = target == 1
    k = int(tpos.sum())
    perm = np.concatenate([np.nonzero(tpos)[0], np.nonzero(~tpos)[0]])
    e2s = emb2[perm]
    e2d = e2s.astype(np.float64)
    e1d = emb1.astype(np.float64)
    b = (e2d * e2d).sum(1) - (2.0 * EPS) * e2d.sum(1)
    a = (e1d * e1d).sum(1) + (2.0 * EPS) * e1d.sum(1) + D * EPS * EPS
    e1tb = np.ascontiguousarray((-2.0 * emb1).T.astype(ml_dtypes.bfloat16))
    e2tb = np.ascontiguousarray(e2s.T.astype(ml_dtypes.bfloat16))
    bhi = b.astype(np.float32).astype(ml_dtypes.bfloat16)
    blo = (b.astype(np.float32) - bhi.astype(np.float32)).astype(ml_dtypes.bfloat16)
    # K=4 tail operands; on device row 4s+r lands at partition 32s+r so the
    # four 512-wide sub-tiles of a group can row-pack on the PE array.
    tailw = np.zeros((16, B), dtype=ml_dtypes.bfloat16)
    trhs = np.zeros((16, B), dtype=ml_dtypes.bfloat16)
    one = np.ones(B, dtype=ml_dtypes.bfloat16)
    for s in range(4):
        tailw[4 * s + 0] = e1tb[126]
        tailw[4 * s + 1] = e1tb[127]
        tailw[4 * s + 2] = one
        tailw[4 * s + 3] = one
        trhs[4 * s + 0] = e2tb[126]
        trhs[4 * s + 1] = e2tb[127]
        trhs[4 * s + 2] = bhi
        trhs[4 * s + 3] = blo
    return k, a, e1tb, e2tb, tailw, trhs, tpos


def _host_finish(a, Mp, mn, tpos, k):
    pos2 = np.clip(a + Mp.astype(np.float64), 0.0, None)
    neg2 = np.clip(a + mn.astype(np.float64), 0.0, None)
    per = np.clip(np.sqrt(pos2) - np.sqrt(neg2) + MARGIN, 0.0, None)
    return np.float32((per * tpos).sum() / k)


def _numpy_fallback(emb1, emb2, target):
    # exact reference recomputation in numpy (degenerate target mixes)
    e1 = emb1.astype(np.float64)
    e2 = emb2.astype(np.float64)
    sq = (
        (e1 * e1).sum(1)[:, None]
        + (e2 * e2).sum(1)[None, :]
        - 2.0 * (e1 @ e2.T)
        + 2.0 * EPS * (e1.sum(1)[:, None] - e2.sum(1)[None, :])
        + D * EPS * EPS
    )
    dist = np.sqrt(np.clip(sq, 0.0, None))
    pos = target == 1
    neg = target == 0
    pos_max = np.where(pos[None, :], dist, -np.inf).max(1)
    neg_min = np.where(neg[None, :], dist, np.inf).min(1)
    per = np.maximum(pos_max - neg_min + MARGIN, 0.0)
    w = pos.astype(np.float64)
    return np.float32((per * w).sum() / w.sum())


def kernel(emb1, emb2, target):
    global LAST_RESULTS
    emb1 = np.asarray(emb1, dtype=np.float32)
    emb2 = np.asarray(emb2, dtype=np.float32)
    target = np.asarray(target)
    assert emb1.shape == (B, D) and emb2.shape == (B, D)

    k = int((target == 1).sum())
    if k == 0 or k == B:
        return _numpy_fallback(emb1, emb2, target)

    k, a, e1tb, e2tb, tailw, trhs, tpos = _host_prep(emb1, emb2, target)

    nc = _programs.get(k)
    if nc is None:
        nc = _build_program(k)
        _programs[k] = nc

    from concourse.bass_utils import run_bass_kernel_spmd

    in_maps = [
        {
            "e1t": np.ascontiguousarray(e1tb[:, c * SH:(c + 1) * SH]),
            "e2t": e2tb,
            "tailw": np.ascontiguousarray(tailw[:, c * SH:(c + 1) * SH]),
            "trhs": trhs,
        }
        for c in range(NCORES)
    ]
    res = run_bass_kernel_spmd(nc, in_maps, core_ids=list(range(NCORES)))
    LAST_RESULTS = res

    Mp = np.concatenate(
        [np.asarray(res.results[c]["out"])[:, :NIT].T.reshape(-1) for c in range(NCORES)]
    )
    mn = np.concatenate(
        [np.asarray(res.results[c]["out"])[:, NIT:].T.reshape(-1) for c in range(NCORES)]
    )
    return _host_finish(a, Mp, mn, tpos, k)
